# revision 32
# baseline (speedup 1.0000x reference)
"""CAB multi-head attention on 8 Trainium2 NeuronCores.

Sharding: fully data-parallel, core c -> (batch b = c//2, query-half = c%2).
Each core computes 256 query rows against all 512 keys of its batch.
No collectives. Host does transposes/packing; device does all FLOPs.

Per-core layout conventions (features on partitions, tokens on free):
  QT/KT [E, t] f32r; V [s, e] f32r; scoresT/attnT [s, t] (softmax along
  partitions via one-hot-column matmuls, no max subtraction needed);
  CAB pairs i-major: h [(d, i%2), j] packed 2 queries per [128, 1024]
  tile; comp [(iic, h, i%2), j] is PE-transposed into the tt-major
  biasT [j, (tt, jc, iic, h, m)] with ONE contiguous scatter copy per
  tt; stage C reads bias via strided 3-dim APs (one per jc).

Engine plan (v3): stage B is elementwise-bound; relu1 runs on the DVE
(bf16 2x mode, 327ns), relu2 on the ACT as [128, 1024] double-tiles
(997ns per 2 queries), the W3 scale and the bias scatter alternate.
Head temps are folded into the W3 weights on the host so the scale op
is a single-ALU-op tensor_scalar.  All V-projection jobs moved into
stage B (PE slack); stage C keeps only the head-10..15 Q/K chunks.
"""
import sys

sys.path.insert(0, "/opt/trn_rl_repo")

import numpy as np
import ml_dtypes
from contextlib import ExitStack

import concourse.bacc as bacc
import concourse.tile as tile
from concourse import mybir
from concourse.bass_utils import run_bass_kernel_spmd

F32 = mybir.dt.float32
F32R = mybir.dt.float32r
BF16 = mybir.dt.bfloat16
AF = mybir.ActivationFunctionType
ALU = mybir.AluOpType

B, N, E, H, SD, HID = 4, 512, 1024, 16, 64, 64
D = E // H
NQ = 256            # query rows per core
NCORES = 8
NTT = NQ // 8       # 32 tt groups (4 i-pairs each) in the CAB stage

_BF = ml_dtypes.bfloat16


def _build_program(debug=False):
    nc = bacc.Bacc("TRN2", target_bir_lowering=False, debug=False,
                   num_devices=NCORES)

    def din(name, shape, dt):
        return nc.dram_tensor(name, list(shape), dt, kind="ExternalInput").ap()

    d = {}
    d["qT"] = din("qT", (E, NQ), BF16)
    d["kT"] = din("kT", (E, N), BF16)
    d["seT"] = din("seT", (SD, N), F32R)
    d["seQ"] = din("seQ", (SD, NQ), F32R)
    # wqP/wkP/vTP are host-packed so each per-job SBUF tile is one
    # contiguous 256KB row-slice (cheap DMA descriptor generation):
    # row ec*128+p, col k*128+c  <-  w[k*128+p, ec*128+c]
    d["wqP"] = din("wqP", (E, E), BF16)
    d["wkP"] = din("wkP", (E, E), BF16)
    d["vTP"] = din("vTP", (N, E), BF16)
    d["wv"] = din("wv", (E, E), BF16)
    d["wo"] = din("wo", (E, E), BF16)
    d["w1a"] = din("w1a", (SD, 128), F32R)
    d["w1b"] = din("w1b", (SD, 128), F32R)
    d["w2bd"] = din("w2bd", (128, 128), BF16)
    d["w3bd"] = din("w3bd", (128, 32), BF16)
    d["id128"] = din("id128", (128, 128), BF16)
    d["hsel"] = din("hsel", (128, 32), BF16)
    d["bq128"] = din("bq128", (128, 8), F32)
    d["bk128"] = din("bk128", (128, 8), F32)
    d["b1d"] = din("b1d", (128, 1), F32)
    d["b2d"] = din("b2d", (128, 1), F32)
    d["b3t"] = din("b3t", (128, 1), F32)
    out_d = nc.dram_tensor("out", [NQ, E], F32, kind="ExternalOutput").ap()
    rscratch = nc.dram_tensor("rscratch", [16, NQ], F32,
                              kind="ExternalOutput").ap()

    with tile.TileContext(nc) as tc, ExitStack() as ctx:
        # ---------------- persistent SBUF pools ----------------
        cst = ctx.enter_context(tc.tile_pool(name="cst", bufs=1))
        big = ctx.enter_context(tc.tile_pool(name="big", bufs=1))

        def cload(name, shape, dt, eng=None):
            eng = eng or nc.sync
            t = cst.tile(list(shape), dt, tag=name, name=name)
            eng.dma_start(t[:], d[name][:])
            return t

        # Startup DMA descriptor generation (~0.7us per dma_start) is spread
        # across all five engine queues so the stage-A/B critical loads
        # aren't serialized behind bulk input loads.
        seT = cload("seT", (SD, N), F32R)                     # sync
        w1b = cload("w1b", (SD, 128), F32R)
        b1d = cload("b1d", (128, 1), F32)
        # prefetch the first 4 k_job weight blocks on the sync queue so the
        # first jobs never head-of-line-block the in-order PE queue
        wkpre = []
        for ec in range(4):
            t = cst.tile([128, 1024], BF16, tag=f"wkpre{ec}", name=f"wkpre{ec}")
            nc.sync.dma_start(t[:], d["wkP"][ec * 128:(ec + 1) * 128, :])
            wkpre.append(t)
        w1a = cload("w1a", (SD, 128), F32R, eng=nc.gpsimd)
        seQ = cload("seQ", (SD, NQ), F32R, eng=nc.gpsimd)
        w2bd = cload("w2bd", (128, 128), BF16, eng=nc.gpsimd)
        b2d = cload("b2d", (128, 1), F32, eng=nc.gpsimd)
        id128 = cload("id128", (128, 128), BF16, eng=nc.gpsimd)
        w3bd = cload("w3bd", (128, 32), BF16, eng=nc.gpsimd)
        b3t = cload("b3t", (128, 1), F32, eng=nc.gpsimd)
        bq128 = cload("bq128", (128, 8), F32, eng=nc.gpsimd)
        bk128 = cload("bk128", (128, 8), F32, eng=nc.gpsimd)
        hsel = cload("hsel", (128, 32), BF16, eng=nc.gpsimd)

        # resident per-core inputs, chunked on k (one DMA each, k-chunk kc
        # of a [E, t] tensor lives in tile kc as [128, t]).
        def kchunks(name, t, dt, ntile=8, eng=None):
            eng = eng or nc.sync
            ts = []
            for k in range(ntile):
                tt = big.tile([128, t], dt, tag=f"{name}{k}", name=f"{name}{k}")
                eng.dma_start(tt[:], d[name][k * 128:(k + 1) * 128, :])
                ts.append(tt)
            return ts

        kTt = kchunks("kT", N, BF16)
        # Wv rows resident (rhs of V-proj), Wo rows resident (rhs of out-proj)
        wv_r = kchunks("wv", E, BF16, eng=nc.gpsimd)
        qTt = kchunks("qT", NQ, BF16, eng=nc.gpsimd)
        # wo is needed only from stage C on; its DMAs are deferred into the
        # tt loop so they don't eat startup HBM bandwidth (which delays the
        # first k_job weights and head-of-line-blocks the PE/ACT queues)
        wo_r = [big.tile([128, E], BF16, tag=f"wo{k}", name=f"wo{k}")
                for k in range(8)]

        # persistent intermediates
        QT = [big.tile([128, NQ], BF16, tag=f"QT{k}", name=f"QT{k}") for k in range(8)]
        KT = [big.tile([128, N], BF16, tag=f"KT{k}", name=f"KT{k}") for k in range(8)]
        Vsb = [[big.tile([128, 512], BF16, tag=f"V{st}_{et}", name=f"V{st}_{et}")
                for et in range(2)] for st in range(4)]
        hjT = big.tile([128, N], BF16, tag="hjT")
        hiT = big.tile([128, 128], F32, tag="hiT")
        # biasT free layout (tt-major): tt*512 + jc*128 + iic*32 + h*2 + m,
        # partition = j within chunk jc.  Written by ONE contiguous copy per
        # tt; stage C reads strided [jc-slices] per (h, half).
        biasT = big.tile([128, NTT * 512], BF16, tag="biasT")
        avN = [big.tile([128, NQ], BF16, tag=f"avN{hp}", name=f"avN{hp}") for hp in range(8)]

        # ---------------- stage A: W1 (tiny) ----------------
        with tc.tile_pool(name="w1ps", bufs=1, space="PSUM") as w1ps:
            hj_ps = w1ps.tile([128, N], F32, tag="hjps")
            nc.tensor.matmul(hj_ps[:], w1b[:], seT[:], start=True, stop=True)
            nc.scalar.activation(hjT[:], hj_ps[:], AF.Identity,
                                 bias=b1d[:, 0:1])
            hi_ps = w1ps.tile([128, NQ], F32, tag="hips")
            nc.tensor.matmul(hi_ps[:], w1a[:], seQ[:], start=True, stop=True)
            hi_v = hi_ps[:].rearrange("p (i two) -> p i two", two=2)
            nc.vector.tensor_copy(hiT[0:64, :], hi_v[0:64, :, 0])
            nc.vector.tensor_copy(hiT[64:128, :], hi_v[64:128, :, 1])

        # ---------------- stage B: CAB pair-MLP + QKV projections ----------
        with tc.tile_pool(name="wcol", bufs=4) as wcol, \
             tc.tile_pool(name="p1ps", bufs=1, space="PSUM") as p1ps, \
             tc.tile_pool(name="hpool", bufs=3) as hpool, \
             tc.tile_pool(name="h2sb", bufs=3) as h2sbp, \
             tc.tile_pool(name="csb", bufs=3) as csbp, \
             tc.tile_pool(name="h2ps", bufs=2, space="PSUM") as h2ps, \
             tc.tile_pool(name="cps", bufs=2, space="PSUM") as cps, \
             tc.tile_pool(name="trps", bufs=1, space="PSUM") as trps:

            # ---- projection jobs, interleaved through the tt loop ----
            # (head-chunks 5-7 of Q/K are only consumed by stage-C pairs 5-7,
            # so those jobs run inside stage C to keep its PE un-throttled)
            def q_job(ec, wpool, pspool):
                wq_c = wpool.tile([128, 1024], BF16, tag="wcol")
                nc.sync.dma_start(
                    wq_c[:], d["wqP"][ec * 128:(ec + 1) * 128, :])
                ps = pspool.tile([128, 512], F32, tag="p1", name="qps")[:, 0:NQ]
                for kc in range(8):
                    nc.tensor.matmul(ps[:], wq_c[:, kc * 128:(kc + 1) * 128],
                                     qTt[kc][:], start=(kc == 0),
                                     stop=(kc == 7))
                nc.vector.tensor_scalar(QT[ec][:], ps[:],
                                        bq128[:, ec:ec + 1], None, ALU.add)

            def k_job(ec, wpool, pspool):
                if ec < 4:
                    wk_c = wkpre[ec]
                else:
                    wk_c = wpool.tile([128, 1024], BF16, tag="wcol")
                    nc.sync.dma_start(
                        wk_c[:], d["wkP"][ec * 128:(ec + 1) * 128, :])
                ps = pspool.tile([128, 512], F32, tag="p1", name="kvps")
                for kc in range(8):
                    nc.tensor.matmul(ps[:], wk_c[:, kc * 128:(kc + 1) * 128],
                                     kTt[kc][:], start=(kc == 0),
                                     stop=(kc == 7))
                nc.scalar.activation(KT[ec][:], ps[:],
                                     AF.Identity, bias=bk128[:, ec:ec + 1])

            def v_job(st, et, wpool=None, pspool=None):
                wpool = wpool or wcol
                pspool = pspool or p1ps
                vt_c = wpool.tile([128, 1024], BF16, tag="vtcb")
                nc.sync.dma_start(
                    vt_c[:], d["vTP"][st * 128:(st + 1) * 128, :])
                ps = pspool.tile([128, 512], F32, tag="p1", name="kvps")
                for kc in range(8):
                    nc.tensor.matmul(
                        ps[:], vt_c[:, kc * 128:(kc + 1) * 128],
                        wv_r[kc][:, et * 512:(et + 1) * 512],
                        start=(kc == 0), stop=(kc == 7))
                if st % 2 == 0:
                    nc.vector.tensor_copy(Vsb[st][et][:], ps[:])
                else:
                    nc.scalar.copy(Vsb[st][et][:], ps[:])

            jobs = ([lambda ec=ec: k_job(ec, wcol, p1ps) for ec in range(5)]
                    + [lambda ec=ec: q_job(ec, wcol, p1ps) for ec in range(5)]
                    + [lambda st=st: v_job(st, 0) for st in range(4)])
            njobs = len(jobs)
            job_i = 0

            for tt in range(NTT):
                if tt == 8:
                    for k in range(8):
                        nc.gpsimd.dma_start(
                            wo_r[k][:], d["wo"][k * 128:(k + 1) * 128, :])
                # spread the 18 projection jobs over tt 3..31 (job weights
                # need a few us of startup DMA bandwidth to arrive)
                while job_i < njobs and job_i < max(0, tt - 2) * njobs // (NTT - 3):
                    jobs[job_i]()
                    job_i += 1

                h2_tiles = []
                for pr in range(2):
                    h_t = hpool.tile([128, 2 * N], BF16, tag="h")
                    for k in range(2):
                        ii = tt * 4 + pr * 2 + k
                        nc.vector.tensor_scalar(h_t[:, k * N:(k + 1) * N],
                                                hjT[:], hiT[:, ii:ii + 1],
                                                0.0, ALU.add, ALU.max)
                    ps = h2ps.tile([128, 2 * N], F32, tag="h2")
                    for k in range(2):
                        nc.tensor.matmul(ps[:, k * N:(k + 1) * N], w2bd[:],
                                         h_t[:, k * N:(k + 1) * N],
                                         start=True, stop=True)
                    h2_t = h2sbp.tile([128, 2 * N], BF16, tag="h2sb")
                    nc.scalar.activation(h2_t[:], ps[:], AF.Relu,
                                         bias=b2d[:, 0:1])
                    h2_tiles.append(h2_t)

                c_ps = cps.tile([128, N], F32, tag="comp")
                for iic in range(4):
                    nc.tensor.matmul(c_ps[32 * iic:32 * iic + 32, :],
                                     w3bd[:],
                                     h2_tiles[iic // 2][:, (iic % 2) * N:
                                                        (iic % 2 + 1) * N],
                                     start=True, stop=True,
                                     tile_position=(0, 32 * iic))
                # temps are folded into w3bd on the host; single add + cast
                c_sb = csbp.tile([128, N], BF16, tag="csb")
                nc.vector.tensor_scalar(c_sb[:], c_ps[:], b3t[:, 0:1],
                                        None, ALU.add)
                tr_ps = trps.tile([128, 512], BF16, tag="tr")
                for jc in range(4):
                    nc.tensor.transpose(tr_ps[:, jc * 128:(jc + 1) * 128],
                                        c_sb[:, jc * 128:(jc + 1) * 128],
                                        id128[:])
                # ONE contiguous scatter copy into the tt-major biasT
                if tt % 2 == 0:
                    nc.scalar.copy(biasT[:, tt * 512:(tt + 1) * 512], tr_ps[:])
                else:
                    nc.vector.tensor_copy(biasT[:, tt * 512:(tt + 1) * 512],
                                          tr_ps[:])

        # ---------------- stage C: scores + softmax + AV ----------------
        # Software-pipelined head loop: head h+1's scores/bias/exp are
        # emitted BEFORE head h's sums/AV matmuls, so the in-order PE queue
        # never stalls on the exp.  scps bufs=4 holds exactly 2 heads.
        # Stage-D partial accumulation for out-block (0,0) replaces the
        # warm_mm filler on the job-less pairs 6-7.
        # bias view: [j-part, (h, jc, tt, iic, m)]
        bT5 = biasT[:].rearrange("p (t j i x m) -> p x j t i m",
                                 t=NTT, j=4, i=4, x=16, m=2)
        with tc.tile_pool(name="attnT", bufs=5) as attp, \
             tc.tile_pool(name="vcol", bufs=4) as vcol, \
             tc.tile_pool(name="vps", bufs=1, space="PSUM") as vps, \
             tc.tile_pool(name="r2sb", bufs=2) as r2sb, \
             tc.tile_pool(name="rc2", bufs=2) as rc2p, \
             tc.tile_pool(name="osb", bufs=2) as osb:

            # per-pair long-matmul jobs: the Q/K projection chunks for heads
            # 10-15 (each needed only from its own pair on)
            cjobs = {
                0: [lambda: k_job(5, vcol, vps), lambda: v_job(0, 1, vcol, vps)],
                1: [lambda: q_job(5, vcol, vps), lambda: v_job(1, 1, vcol, vps)],
                2: [lambda: k_job(6, vcol, vps), lambda: v_job(2, 1, vcol, vps)],
                3: [lambda: q_job(6, vcol, vps), lambda: v_job(3, 1, vcol, vps)],
                4: [lambda: k_job(7, vcol, vps)],
                5: [lambda: q_job(7, vcol, vps)],
            }
            dps_t = None

            # scps bufs=5 gives ~3 halves of PE lookahead; the softmax sums
            # share the av PSUM bank (avsums cols 256:512 on partitions 0:2)
            # so only one av/sums tile per pair is needed.  The pair's very
            # first AV matmul carries the lone start=True (clears the bank's
            # has_written bits); every other matmul overwrites/accumulates
            # via those bits in program order.
            with tc.tile_pool(name="scps", bufs=4, space="PSUM") as scps, \
                 tc.tile_pool(name="smps", bufs=1, space="PSUM") as smps, \
                 tc.tile_pool(name="avps", bufs=2, space="PSUM") as avps:

                def emit_half(h, half):
                    hp, hw = h // 2, (h % 2) * 64
                    sc_ps = scps.tile([128, 512], F32, tag="sc")
                    for q in range(2):
                        jc = half * 2 + q
                        nc.tensor.matmul(
                            sc_ps[:, q * 256:(q + 1) * 256],
                            KT[hp][hw:hw + 64, jc * 128:(jc + 1) * 128],
                            QT[hp][hw:hw + 64, :],
                            start=True, stop=True, skip_group_check=True)
                    for q in range(2):
                        jc = half * 2 + q
                        nc.vector.tensor_tensor(
                            sc_ps[:, q * 256:(q + 1) * 256],
                            sc_ps[:, q * 256:(q + 1) * 256],
                            bT5[:, h, jc], ALU.add)
                    at = attp.tile([128, 512], BF16, tag="at")
                    nc.scalar.activation(at[:], sc_ps[:], AF.Exp)
                    return at

                LOOK = 3
                pend = {}
                avsums = None
                sums2 = None
                for idx in range(32 + LOOK):
                    if idx < 32:
                        h, half = idx // 2, idx % 2
                        if idx % 4 == 0:
                            hp = h // 2
                            for job in cjobs.get(hp, []):
                                job()
                            if hp == 6:
                                dps_t = vps.tile([128, 512], F32, tag="p1",
                                                 name="dpart")
                                for ihp in range(4):
                                    nc.tensor.matmul(
                                        dps_t[:], avN[ihp][:, 0:128],
                                        wo_r[ihp][:, 0:512], start=(ihp == 0),
                                        stop=False, skip_group_check=True)
                            elif hp == 7:
                                for ihp in (4, 5):
                                    nc.tensor.matmul(
                                        dps_t[:], avN[ihp][:, 0:128],
                                        wo_r[ihp][:, 0:512], start=False,
                                        stop=False, skip_group_check=True)
                        pend[idx] = emit_half(h, half)
                    j = idx - LOOK
                    if j < 0:
                        continue
                    h, half = j // 2, j % 2
                    hp, hw = h // 2, (h % 2) * 64
                    if j % 4 == 0:
                        avsums = avps.tile([128, NQ], F32, tag="av")
                        sums2 = smps.tile([2, NQ], F32, tag="s2")
                    ats = pend.pop(j)
                    for q in range(2):
                        jc = half * 2 + q
                        atv = ats[:, q * 256:(q + 1) * 256]
                        nc.tensor.matmul(
                            avsums[hw:hw + 64, :],
                            Vsb[jc][h // 8][:, (h % 8) * 64:(h % 8) * 64 + 64],
                            atv,
                            start=(half == 0 and q == 0),
                            stop=(half == 1 and q == 1),
                            skip_group_check=True,
                            tile_position=(0, hw))
                    for q in range(2):
                        jc = half * 2 + q
                        atv = ats[:, q * 256:(q + 1) * 256]
                        nc.tensor.matmul(
                            sums2[:], hsel[:, 2 * h:2 * h + 2], atv,
                            start=(j % 4 == 0 and q == 0),
                            stop=(j % 4 == 3 and q == 1),
                            skip_group_check=True)
                    if j % 4 == 3:
                        recip2 = rc2p.tile([2, NQ], F32, tag="rc2")
                        nc.vector.reciprocal_approx_fast(recip2[:], sums2[:])
                        nc.sync.dma_start(rscratch[2 * hp:2 * hp + 2, :],
                                          recip2[:])
                        r2 = r2sb.tile([128, NQ], F32, tag="r2")
                        rsrc = rscratch[2 * hp:2 * hp + 2, :].rearrange(
                            "h (o t) -> h o t", o=1)
                        nc.sync.dma_start(r2[:], rsrc.broadcast_to([2, 64, NQ]))
                        nc.vector.tensor_tensor(avN[hp][:], avsums[:],
                                                r2[:], ALU.mult)

            # ------------- stage D: output projection -------------
            # (scps/smps/avps banks are freed; block (0,0) finishes on the
            # vps bank it accumulated into during pairs 6-7)
            with tc.tile_pool(name="ops", bufs=2, space="PSUM") as ops:
                for ihp in (6, 7):
                    nc.tensor.matmul(dps_t[:], avN[ihp][:, 0:128],
                                     wo_r[ihp][:, 0:512], start=False,
                                     stop=(ihp == 7), skip_group_check=True)
                o_sb = osb.tile([128, 512], F32, tag="osb")
                nc.scalar.copy(o_sb[:], dps_t[:])
                nc.sync.dma_start(out_d[0:128, 0:512], o_sb[:])
                for ttile, et in ((0, 1), (1, 0), (1, 1)):
                    ps = ops.tile([128, 512], F32, tag="ops")
                    for hp in range(8):
                        nc.tensor.matmul(
                            ps[:], avN[hp][:, ttile * 128:(ttile + 1) * 128],
                            wo_r[hp][:, et * 512:(et + 1) * 512],
                            start=(hp == 0), stop=(hp == 7))
                    o_sb = osb.tile([128, 512], F32, tag="osb")
                    nc.scalar.copy(o_sb[:], ps[:])
                    nc.sync.dma_start(
                        out_d[ttile * 128:(ttile + 1) * 128,
                              et * 512:(et + 1) * 512], o_sb[:])

    nc.compile()
    return nc


def _host_prep(inputs):
    """Build the 8 per-core input maps from the full inputs."""
    f32 = np.float32
    q = np.ascontiguousarray(inputs["query"], f32)
    k = np.ascontiguousarray(inputs["key"], f32)
    v = np.ascontiguousarray(inputs["value"], f32)
    se = np.ascontiguousarray(inputs["state_embeddings"], f32)
    scale = f32(D) ** f32(-0.5)
    # packed layouts: row ec*128+p, col k*128+c  <-  w[k*128+p, ec*128+c]
    def pack_w(w, necol):
        return np.ascontiguousarray(
            w.reshape(8, 128, necol, 128).transpose(2, 1, 0, 3)
            .reshape(necol * 128, 1024))
    wqP = pack_w(np.asarray(inputs["Wq"], f32) * scale, 8).astype(_BF)
    wkP = pack_w(np.asarray(inputs["Wk"], f32), 8).astype(_BF)
    wv = np.ascontiguousarray(inputs["Wv"]).astype(_BF)
    wo = np.ascontiguousarray(inputs["Wo"]).astype(_BF)
    bq = np.asarray(inputs["bq"], f32) * scale
    bk = np.asarray(inputs["bk"], f32)
    w1 = np.asarray(inputs["W1"], f32)
    b1 = np.asarray(inputs["b1"], f32)
    w2 = np.asarray(inputs["W2"], f32)
    b2 = np.asarray(inputs["b2"], f32)
    w3 = np.asarray(inputs["W3"], f32)
    b3 = np.asarray(inputs["b3"], f32)
    temps = np.asarray(inputs["head_temps"], f32)

    w1a_dup = np.concatenate([w1[:SD], w1[:SD]], axis=1)          # [64,128]
    w1b_dup = np.concatenate([w1[SD:], w1[SD:]], axis=1)          # [64,128]
    w2bd = np.zeros((128, 128), f32)
    w2bd[:64, :64] = w2
    w2bd[64:, 64:] = w2
    # head temps folded into the W3 columns (m = 2*h + par, h-major pairs)
    w3t = w3 * temps[None, :]
    w3bd = np.zeros((128, 32), f32)
    w3bd[:64, 0::2] = w3t
    w3bd[64:, 1::2] = w3t
    hsel = np.zeros((128, 32), f32)
    for h in range(H):
        hsel[:, 2 * h + h % 2] = 1.0
    hidx = (np.arange(128) % 32) // 2
    b3t = (b3 * temps)[hidx].reshape(128, 1)
    b1d = np.tile(b1, 2).reshape(128, 1)
    b2d = np.tile(b2, 2).reshape(128, 1)
    bq128 = bq.reshape(8, 128).T.copy()
    bk128 = bk.reshape(8, 128).T.copy()
    id128 = np.eye(128, dtype=f32).astype(_BF)

    shared = dict(wqP=wqP, wkP=wkP, wv=wv, wo=wo, w1a=w1a_dup, w1b=w1b_dup,
                  w2bd=w2bd.astype(_BF), w3bd=w3bd.astype(_BF),
                  id128=id128, hsel=hsel.astype(_BF), bq128=bq128, bk128=bk128,
                  b1d=b1d, b2d=b2d, b3t=b3t)
    maps = []
    for c in range(NCORES):
        b, half = c // 2, c % 2
        rows = slice(half * NQ, (half + 1) * NQ)
        m = dict(shared)
        m["qT"] = np.ascontiguousarray(q[b, rows].T).astype(_BF)
        m["kT"] = np.ascontiguousarray(k[b].T).astype(_BF)
        m["vTP"] = pack_w(np.ascontiguousarray(v[b].T), 4).astype(_BF)
        m["seT"] = np.ascontiguousarray(se[b].T)
        m["seQ"] = np.ascontiguousarray(se[b, rows].T)
        maps.append(m)
    return maps


_cache = {}


def _get_program():
    if "nc" not in _cache:
        _cache["nc"] = _build_program()
    return _cache["nc"]


def kernel(**inputs):
    nc = _get_program()
    maps = _host_prep(inputs)
    res = run_bass_kernel_spmd(nc, maps, list(range(NCORES)))
    # attention weights sum to 1, so bv contributes exactly bv @ Wo
    bo = (np.asarray(inputs["bo"], np.float32)
          + np.asarray(inputs["bv"], np.float32)
          @ np.asarray(inputs["Wo"], np.float32))
    out = np.empty((B, N, E), np.float32)
    for c in range(NCORES):
        b, half = c // 2, c % 2
        out[b, half * NQ:(half + 1) * NQ] = res.results[c]["out"]
    return out + bo


# revision 35
# speedup vs baseline: 1.1319x; 1.1319x over previous
"""CAB multi-head attention on 8 Trainium2 NeuronCores.

Sharding: fully data-parallel, core c -> (batch b = c//2, query-half = c%2).
Each core computes 256 query rows against all 512 keys of its batch.
No collectives. Host does transposes/packing; device does all FLOPs.

Per-core layout conventions (features on partitions, tokens on free):
  QT/KT [E, t] f32r; V [s, e] f32r; scoresT/attnT [s, t] (softmax along
  partitions via one-hot-column matmuls, no max subtraction needed);
  CAB pairs i-major: h [(d, i%2), j] packed 2 queries per [128, 1024]
  tile; comp [(iic, h, i%2), j] is PE-transposed into the tt-major
  biasT [j, (tt, jc, iic, h, m)] with ONE contiguous scatter copy per
  tt; stage C reads bias via strided 3-dim APs (one per jc).

Engine plan (v3): stage B is elementwise-bound; relu1 runs on the DVE
(bf16 2x mode, 327ns), relu2 on the ACT as [128, 1024] double-tiles
(997ns per 2 queries), the W3 scale and the bias scatter alternate.
Head temps are folded into the W3 weights on the host so the scale op
is a single-ALU-op tensor_scalar.  All V-projection jobs moved into
stage B (PE slack); stage C keeps only the head-10..15 Q/K chunks.
"""
import sys

sys.path.insert(0, "/opt/trn_rl_repo")

import numpy as np
import ml_dtypes
from contextlib import ExitStack

import concourse.bacc as bacc
import concourse.tile as tile
from concourse import mybir
from concourse.bass_utils import run_bass_kernel_spmd

F32 = mybir.dt.float32
F32R = mybir.dt.float32r
BF16 = mybir.dt.bfloat16
AF = mybir.ActivationFunctionType
ALU = mybir.AluOpType

B, N, E, H, SD, HID = 4, 512, 1024, 16, 64, 64
D = E // H
NQ = 256            # query rows per core
NCORES = 8
NTT = NQ // 8       # 32 tt groups (4 i-pairs each) in the CAB stage

_BF = ml_dtypes.bfloat16


def _build_program(debug=False):
    nc = bacc.Bacc("TRN2", target_bir_lowering=False, debug=False,
                   num_devices=NCORES)

    def din(name, shape, dt):
        return nc.dram_tensor(name, list(shape), dt, kind="ExternalInput").ap()

    d = {}
    d["qT"] = din("qT", (E, NQ), BF16)
    d["kT"] = din("kT", (E, N), BF16)
    d["seT"] = din("seT", (SD, N), F32R)
    d["seQ"] = din("seQ", (SD, NQ), F32R)
    # wqP/wkP/vTP are host-packed so each per-job SBUF tile is one
    # contiguous 256KB row-slice (cheap DMA descriptor generation):
    # row ec*128+p, col k*128+c  <-  w[k*128+p, ec*128+c]
    d["wqP"] = din("wqP", (E, E), BF16)
    d["wkP"] = din("wkP", (E, E), BF16)
    d["vTP"] = din("vTP", (N, E), BF16)
    d["wv"] = din("wv", (E, E), BF16)
    d["wo"] = din("wo", (E, E), BF16)
    d["w1a"] = din("w1a", (SD, 128), F32R)
    d["w1b"] = din("w1b", (SD, 128), F32R)
    d["w2bd"] = din("w2bd", (128, 128), BF16)
    d["w3bd"] = din("w3bd", (128, 32), BF16)
    d["id128"] = din("id128", (128, 128), BF16)
    d["hsel"] = din("hsel", (128, 32), BF16)
    d["bq128"] = din("bq128", (128, 8), F32)
    d["bk128"] = din("bk128", (128, 8), F32)
    d["b1d"] = din("b1d", (128, 1), F32)
    d["b2d"] = din("b2d", (128, 1), F32)
    d["b3t"] = din("b3t", (128, 1), F32)
    out_d = nc.dram_tensor("out", [NQ, E], F32, kind="ExternalOutput").ap()
    rscratch = nc.dram_tensor("rscratch", [16, NQ], F32,
                              kind="ExternalOutput").ap()

    with tile.TileContext(nc) as tc, ExitStack() as ctx:
        # ---------------- persistent SBUF pools ----------------
        cst = ctx.enter_context(tc.tile_pool(name="cst", bufs=1))
        big = ctx.enter_context(tc.tile_pool(name="big", bufs=1))

        def cload(name, shape, dt, eng=None):
            eng = eng or nc.sync
            t = cst.tile(list(shape), dt, tag=name, name=name)
            eng.dma_start(t[:], d[name][:])
            return t

        # Startup DMA descriptor generation (~0.7us per dma_start) is spread
        # across all five engine queues so the stage-A/B critical loads
        # aren't serialized behind bulk input loads.
        seT = cload("seT", (SD, N), F32R)                     # sync
        w1b = cload("w1b", (SD, 128), F32R)
        b1d = cload("b1d", (128, 1), F32)
        # prefetch the first 4 k_job weight blocks on the sync queue so the
        # first jobs never head-of-line-block the in-order PE queue
        wkpre = []
        for ec in range(4):
            t = cst.tile([128, 1024], BF16, tag=f"wkpre{ec}", name=f"wkpre{ec}")
            nc.sync.dma_start(t[:], d["wkP"][ec * 128:(ec + 1) * 128, :])
            wkpre.append(t)
        w1a = cload("w1a", (SD, 128), F32R, eng=nc.gpsimd)
        seQ = cload("seQ", (SD, NQ), F32R, eng=nc.gpsimd)
        w2bd = cload("w2bd", (128, 128), BF16, eng=nc.gpsimd)
        b2d = cload("b2d", (128, 1), F32, eng=nc.gpsimd)
        id128 = cload("id128", (128, 128), BF16, eng=nc.gpsimd)
        w3bd = cload("w3bd", (128, 32), BF16, eng=nc.gpsimd)
        b3t = cload("b3t", (128, 1), F32, eng=nc.gpsimd)
        bq128 = cload("bq128", (128, 8), F32, eng=nc.gpsimd)
        bk128 = cload("bk128", (128, 8), F32, eng=nc.gpsimd)
        hsel = cload("hsel", (128, 32), BF16, eng=nc.gpsimd)

        # resident per-core inputs, chunked on k (one DMA each, k-chunk kc
        # of a [E, t] tensor lives in tile kc as [128, t]).
        def kchunks(name, t, dt, ntile=8, eng=None):
            eng = eng or nc.sync
            ts = []
            for k in range(ntile):
                tt = big.tile([128, t], dt, tag=f"{name}{k}", name=f"{name}{k}")
                eng.dma_start(tt[:], d[name][k * 128:(k + 1) * 128, :])
                ts.append(tt)
            return ts

        kTt = kchunks("kT", N, BF16)
        # Wv rows resident (rhs of V-proj), Wo rows resident (rhs of out-proj)
        wv_r = kchunks("wv", E, BF16, eng=nc.gpsimd)
        qTt = kchunks("qT", NQ, BF16, eng=nc.gpsimd)
        # wo is needed only from stage C on; its DMAs are deferred into the
        # tt loop so they don't eat startup HBM bandwidth (which delays the
        # first k_job weights and head-of-line-blocks the PE/ACT queues)
        wo_r = [big.tile([128, E], BF16, tag=f"wo{k}", name=f"wo{k}")
                for k in range(8)]

        # persistent intermediates
        QT = [big.tile([128, NQ], BF16, tag=f"QT{k}", name=f"QT{k}") for k in range(8)]
        KT = [big.tile([128, N], BF16, tag=f"KT{k}", name=f"KT{k}") for k in range(8)]
        Vsb = [[big.tile([128, 512], BF16, tag=f"V{st}_{et}", name=f"V{st}_{et}")
                for et in range(2)] for st in range(4)]
        hjT = big.tile([128, N], BF16, tag="hjT")
        hiT = big.tile([128, 128], F32, tag="hiT")
        # biasT free layout (tt-major): tt*512 + jc*128 + iic*32 + h*2 + m,
        # partition = j within chunk jc.  Written by ONE contiguous copy per
        # tt; stage C reads strided [jc-slices] per (h, half).
        biasT = big.tile([128, NTT * 512], BF16, tag="biasT")
        avN = [big.tile([128, NQ], BF16, tag=f"avN{hp}", name=f"avN{hp}") for hp in range(8)]

        # ---------------- stage A: W1 (tiny) ----------------
        with tc.tile_pool(name="w1ps", bufs=1, space="PSUM") as w1ps:
            hj_ps = w1ps.tile([128, N], F32, tag="hjps")
            nc.tensor.matmul(hj_ps[:], w1b[:], seT[:], start=True, stop=True)
            nc.scalar.activation(hjT[:], hj_ps[:], AF.Identity,
                                 bias=b1d[:, 0:1])
            hi_ps = w1ps.tile([128, NQ], F32, tag="hips")
            nc.tensor.matmul(hi_ps[:], w1a[:], seQ[:], start=True, stop=True)
            hi_v = hi_ps[:].rearrange("p (i two) -> p i two", two=2)
            nc.vector.tensor_copy(hiT[0:64, :], hi_v[0:64, :, 0])
            nc.vector.tensor_copy(hiT[64:128, :], hi_v[64:128, :, 1])

        # ---------------- stage B: CAB pair-MLP + QKV projections ----------
        with tc.tile_pool(name="wcol", bufs=4) as wcol, \
             tc.tile_pool(name="p1ps", bufs=1, space="PSUM") as p1ps, \
             tc.tile_pool(name="hpool", bufs=3) as hpool, \
             tc.tile_pool(name="h2sb", bufs=3) as h2sbp, \
             tc.tile_pool(name="csb", bufs=3) as csbp, \
             tc.tile_pool(name="h2ps", bufs=2, space="PSUM") as h2ps, \
             tc.tile_pool(name="cps", bufs=2, space="PSUM") as cps, \
             tc.tile_pool(name="trps", bufs=1, space="PSUM") as trps:

            # ---- projection jobs, interleaved through the tt loop ----
            # (head-chunks 5-7 of Q/K are only consumed by stage-C pairs 5-7,
            # so those jobs run inside stage C to keep its PE un-throttled)
            def q_job(ec, wpool, pspool):
                wq_c = wpool.tile([128, 1024], BF16, tag="wcol")
                nc.sync.dma_start(
                    wq_c[:], d["wqP"][ec * 128:(ec + 1) * 128, :])
                ps = pspool.tile([128, 512], F32, tag="p1", name="qps")[:, 0:NQ]
                for kc in range(8):
                    nc.tensor.matmul(ps[:], wq_c[:, kc * 128:(kc + 1) * 128],
                                     qTt[kc][:], start=(kc == 0),
                                     stop=(kc == 7))
                nc.vector.tensor_scalar(QT[ec][:], ps[:],
                                        bq128[:, ec:ec + 1], None, ALU.add)

            def k_job(ec, wpool, pspool):
                if ec < 4:
                    wk_c = wkpre[ec]
                else:
                    wk_c = wpool.tile([128, 1024], BF16, tag="wcol")
                    nc.sync.dma_start(
                        wk_c[:], d["wkP"][ec * 128:(ec + 1) * 128, :])
                ps = pspool.tile([128, 512], F32, tag="p1", name="kvps")
                for kc in range(8):
                    nc.tensor.matmul(ps[:], wk_c[:, kc * 128:(kc + 1) * 128],
                                     kTt[kc][:], start=(kc == 0),
                                     stop=(kc == 7))
                nc.scalar.activation(KT[ec][:], ps[:],
                                     AF.Identity, bias=bk128[:, ec:ec + 1])

            def v_job(st, et, wpool=None, pspool=None):
                wpool = wpool or wcol
                pspool = pspool or p1ps
                vt_c = wpool.tile([128, 1024], BF16, tag="vtcb")
                nc.sync.dma_start(
                    vt_c[:], d["vTP"][st * 128:(st + 1) * 128, :])
                ps = pspool.tile([128, 512], F32, tag="p1", name="kvps")
                for kc in range(8):
                    nc.tensor.matmul(
                        ps[:], vt_c[:, kc * 128:(kc + 1) * 128],
                        wv_r[kc][:, et * 512:(et + 1) * 512],
                        start=(kc == 0), stop=(kc == 7))
                if st % 2 == 0:
                    nc.vector.tensor_copy(Vsb[st][et][:], ps[:])
                else:
                    nc.scalar.copy(Vsb[st][et][:], ps[:])

            jobs = ([lambda ec=ec: k_job(ec, wcol, p1ps) for ec in range(5)]
                    + [lambda ec=ec: q_job(ec, wcol, p1ps) for ec in range(5)]
                    + [lambda st=st: v_job(st, 0) for st in range(4)]
                    + [lambda st=st: v_job(st, 1) for st in range(4)])
            njobs = len(jobs)
            job_i = 0

            for tt in range(NTT):
                if tt == 8:
                    for k in range(8):
                        nc.gpsimd.dma_start(
                            wo_r[k][:], d["wo"][k * 128:(k + 1) * 128, :])
                # spread the 18 projection jobs over tt 3..31 (job weights
                # need a few us of startup DMA bandwidth to arrive)
                while job_i < njobs and job_i < max(0, tt - 2) * njobs // (NTT - 3):
                    jobs[job_i]()
                    job_i += 1

                h2_tiles = []
                for pr in range(2):
                    h_t = hpool.tile([128, 2 * N], BF16, tag="h")
                    for k in range(2):
                        ii = tt * 4 + pr * 2 + k
                        nc.vector.tensor_scalar(h_t[:, k * N:(k + 1) * N],
                                                hjT[:], hiT[:, ii:ii + 1],
                                                0.0, ALU.add, ALU.max)
                    ps = h2ps.tile([128, 2 * N], F32, tag="h2")
                    for k in range(2):
                        nc.tensor.matmul(ps[:, k * N:(k + 1) * N], w2bd[:],
                                         h_t[:, k * N:(k + 1) * N],
                                         start=True, stop=True)
                    h2_t = h2sbp.tile([128, 2 * N], BF16, tag="h2sb")
                    nc.scalar.activation(h2_t[:], ps[:], AF.Relu,
                                         bias=b2d[:, 0:1])
                    h2_tiles.append(h2_t)

                c_ps = cps.tile([128, N], F32, tag="comp")
                for iic in range(4):
                    nc.tensor.matmul(c_ps[32 * iic:32 * iic + 32, :],
                                     w3bd[:],
                                     h2_tiles[iic // 2][:, (iic % 2) * N:
                                                        (iic % 2 + 1) * N],
                                     start=True, stop=True,
                                     tile_position=(0, 32 * iic))
                # temps are folded into w3bd on the host; single add + cast
                c_sb = csbp.tile([128, N], BF16, tag="csb")
                nc.vector.tensor_scalar(c_sb[:], c_ps[:], b3t[:, 0:1],
                                        None, ALU.add)
                tr_ps = trps.tile([128, 512], BF16, tag="tr")
                for jc in range(4):
                    nc.tensor.transpose(tr_ps[:, jc * 128:(jc + 1) * 128],
                                        c_sb[:, jc * 128:(jc + 1) * 128],
                                        id128[:])
                # ONE contiguous scatter copy into the tt-major biasT
                if tt % 2 == 0:
                    nc.scalar.copy(biasT[:, tt * 512:(tt + 1) * 512], tr_ps[:])
                else:
                    nc.vector.tensor_copy(biasT[:, tt * 512:(tt + 1) * 512],
                                          tr_ps[:])

        # ---------------- stage C: scores + softmax + AV ----------------
        # Software-pipelined head loop: head h+1's scores/bias/exp are
        # emitted BEFORE head h's sums/AV matmuls, so the in-order PE queue
        # never stalls on the exp.  scps bufs=4 holds exactly 2 heads.
        # Stage-D partial accumulation for out-block (0,0) replaces the
        # warm_mm filler on the job-less pairs 6-7.
        # bias view: [j-part, (h, jc, tt, iic, m)]
        bT5 = biasT[:].rearrange("p (t j i x m) -> p x j t i m",
                                 t=NTT, j=4, i=4, x=16, m=2)
        with tc.tile_pool(name="attnT", bufs=5) as attp, \
             tc.tile_pool(name="vcol", bufs=4) as vcol, \
             tc.tile_pool(name="vps", bufs=1, space="PSUM") as vps, \
             tc.tile_pool(name="r2sb", bufs=2) as r2sb, \
             tc.tile_pool(name="rc2", bufs=2) as rc2p, \
             tc.tile_pool(name="osb", bufs=2) as osb:

            # per-pair long-matmul jobs: the Q/K projection chunks for heads
            # 10-15 (each needed only from its own pair on)
            cjobs = {
                0: [lambda: k_job(5, vcol, vps)],
                1: [lambda: q_job(5, vcol, vps)],
                2: [lambda: k_job(6, vcol, vps)],
                3: [lambda: q_job(6, vcol, vps)],
                4: [lambda: k_job(7, vcol, vps)],
                5: [lambda: q_job(7, vcol, vps)],
            }
            dps_t = None

            # scps bufs=5 gives ~3 halves of PE lookahead; the softmax sums
            # share the av PSUM bank (avsums cols 256:512 on partitions 0:2)
            # so only one av/sums tile per pair is needed.  The pair's very
            # first AV matmul carries the lone start=True (clears the bank's
            # has_written bits); every other matmul overwrites/accumulates
            # via those bits in program order.
            with tc.tile_pool(name="scps", bufs=4, space="PSUM") as scps, \
                 tc.tile_pool(name="smps", bufs=1, space="PSUM") as smps, \
                 tc.tile_pool(name="avps", bufs=2, space="PSUM") as avps:

                def emit_half(h, half):
                    hp, hw = h // 2, (h % 2) * 64
                    sc_ps = scps.tile([128, 512], F32, tag="sc")
                    for q in range(2):
                        jc = half * 2 + q
                        nc.tensor.matmul(
                            sc_ps[:, q * 256:(q + 1) * 256],
                            KT[hp][hw:hw + 64, jc * 128:(jc + 1) * 128],
                            QT[hp][hw:hw + 64, :],
                            start=True, stop=True, skip_group_check=True)
                    for q in range(2):
                        jc = half * 2 + q
                        nc.vector.tensor_tensor(
                            sc_ps[:, q * 256:(q + 1) * 256],
                            sc_ps[:, q * 256:(q + 1) * 256],
                            bT5[:, h, jc], ALU.add)
                    at = attp.tile([128, 512], BF16, tag="at")
                    nc.scalar.activation(at[:], sc_ps[:], AF.Exp)
                    return at

                LOOK = 2
                pend = {}
                avsums = None
                sums2 = None
                for idx in range(32 + LOOK):
                    if idx < 32:
                        h, half = idx // 2, idx % 2
                        if idx % 4 == 0:
                            hp = h // 2
                            for job in cjobs.get(hp, []):
                                job()
                            if hp == 6:
                                dps_t = vps.tile([128, 512], F32, tag="p1",
                                                 name="dpart")
                                for ihp in range(4):
                                    nc.tensor.matmul(
                                        dps_t[:], avN[ihp][:, 0:128],
                                        wo_r[ihp][:, 0:512], start=(ihp == 0),
                                        stop=False, skip_group_check=True)
                            elif hp == 7:
                                for ihp in (4, 5):
                                    nc.tensor.matmul(
                                        dps_t[:], avN[ihp][:, 0:128],
                                        wo_r[ihp][:, 0:512], start=False,
                                        stop=False, skip_group_check=True)
                        pend[idx] = emit_half(h, half)
                    j = idx - LOOK
                    if j < 0:
                        continue
                    h, half = j // 2, j % 2
                    hp, hw = h // 2, (h % 2) * 64
                    if j % 4 == 0:
                        avsums = avps.tile([128, NQ], F32, tag="av")
                        sums2 = smps.tile([2, NQ], F32, tag="s2")
                    ats = pend.pop(j)
                    for q in range(2):
                        jc = half * 2 + q
                        atv = ats[:, q * 256:(q + 1) * 256]
                        nc.tensor.matmul(
                            avsums[hw:hw + 64, :],
                            Vsb[jc][h // 8][:, (h % 8) * 64:(h % 8) * 64 + 64],
                            atv,
                            start=(half == 0 and q == 0),
                            stop=(half == 1 and q == 1),
                            skip_group_check=True,
                            tile_position=(0, hw))
                    for q in range(2):
                        jc = half * 2 + q
                        atv = ats[:, q * 256:(q + 1) * 256]
                        nc.tensor.matmul(
                            sums2[:], hsel[:, 2 * h:2 * h + 2], atv,
                            start=(j % 4 == 0 and q == 0),
                            stop=(j % 4 == 3 and q == 1),
                            skip_group_check=True)
                    if j % 4 == 3:
                        recip2 = rc2p.tile([2, NQ], F32, tag="rc2")
                        nc.vector.reciprocal_approx_fast(recip2[:], sums2[:])
                        nc.sync.dma_start(rscratch[2 * hp:2 * hp + 2, :],
                                          recip2[:])
                        r2 = r2sb.tile([128, NQ], F32, tag="r2")
                        rsrc = rscratch[2 * hp:2 * hp + 2, :].rearrange(
                            "h (o t) -> h o t", o=1)
                        nc.sync.dma_start(r2[:], rsrc.broadcast_to([2, 64, NQ]))
                        nc.vector.tensor_tensor(avN[hp][:], avsums[:],
                                                r2[:], ALU.mult)

            # ------------- stage D: output projection -------------
            # (scps/smps/avps banks are freed; block (0,0) finishes on the
            # vps bank it accumulated into during pairs 6-7)
            with tc.tile_pool(name="ops", bufs=2, space="PSUM") as ops:
                for ihp in (6, 7):
                    nc.tensor.matmul(dps_t[:], avN[ihp][:, 0:128],
                                     wo_r[ihp][:, 0:512], start=False,
                                     stop=(ihp == 7), skip_group_check=True)
                o_sb = osb.tile([128, 512], F32, tag="osb")
                nc.scalar.copy(o_sb[:], dps_t[:])
                nc.sync.dma_start(out_d[0:128, 0:512], o_sb[:])
                for ttile, et in ((0, 1), (1, 0), (1, 1)):
                    ps = ops.tile([128, 512], F32, tag="ops")
                    for hp in range(8):
                        nc.tensor.matmul(
                            ps[:], avN[hp][:, ttile * 128:(ttile + 1) * 128],
                            wo_r[hp][:, et * 512:(et + 1) * 512],
                            start=(hp == 0), stop=(hp == 7))
                    o_sb = osb.tile([128, 512], F32, tag="osb")
                    nc.scalar.copy(o_sb[:], ps[:])
                    nc.sync.dma_start(
                        out_d[ttile * 128:(ttile + 1) * 128,
                              et * 512:(et + 1) * 512], o_sb[:])

    nc.compile()
    return nc


def _host_prep(inputs):
    """Build the 8 per-core input maps from the full inputs."""
    f32 = np.float32
    q = np.ascontiguousarray(inputs["query"], f32)
    k = np.ascontiguousarray(inputs["key"], f32)
    v = np.ascontiguousarray(inputs["value"], f32)
    se = np.ascontiguousarray(inputs["state_embeddings"], f32)
    scale = f32(D) ** f32(-0.5)
    # packed layouts: row ec*128+p, col k*128+c  <-  w[k*128+p, ec*128+c]
    def pack_w(w, necol):
        return np.ascontiguousarray(
            w.reshape(8, 128, necol, 128).transpose(2, 1, 0, 3)
            .reshape(necol * 128, 1024))
    wqP = pack_w(np.asarray(inputs["Wq"], f32) * scale, 8).astype(_BF)
    wkP = pack_w(np.asarray(inputs["Wk"], f32), 8).astype(_BF)
    wv = np.ascontiguousarray(inputs["Wv"]).astype(_BF)
    wo = np.ascontiguousarray(inputs["Wo"]).astype(_BF)
    bq = np.asarray(inputs["bq"], f32) * scale
    bk = np.asarray(inputs["bk"], f32)
    w1 = np.asarray(inputs["W1"], f32)
    b1 = np.asarray(inputs["b1"], f32)
    w2 = np.asarray(inputs["W2"], f32)
    b2 = np.asarray(inputs["b2"], f32)
    w3 = np.asarray(inputs["W3"], f32)
    b3 = np.asarray(inputs["b3"], f32)
    temps = np.asarray(inputs["head_temps"], f32)

    w1a_dup = np.concatenate([w1[:SD], w1[:SD]], axis=1)          # [64,128]
    w1b_dup = np.concatenate([w1[SD:], w1[SD:]], axis=1)          # [64,128]
    w2bd = np.zeros((128, 128), f32)
    w2bd[:64, :64] = w2
    w2bd[64:, 64:] = w2
    # head temps folded into the W3 columns (m = 2*h + par, h-major pairs)
    w3t = w3 * temps[None, :]
    w3bd = np.zeros((128, 32), f32)
    w3bd[:64, 0::2] = w3t
    w3bd[64:, 1::2] = w3t
    hsel = np.zeros((128, 32), f32)
    for h in range(H):
        hsel[:, 2 * h + h % 2] = 1.0
    hidx = (np.arange(128) % 32) // 2
    b3t = (b3 * temps)[hidx].reshape(128, 1)
    b1d = np.tile(b1, 2).reshape(128, 1)
    b2d = np.tile(b2, 2).reshape(128, 1)
    bq128 = bq.reshape(8, 128).T.copy()
    bk128 = bk.reshape(8, 128).T.copy()
    id128 = np.eye(128, dtype=f32).astype(_BF)

    shared = dict(wqP=wqP, wkP=wkP, wv=wv, wo=wo, w1a=w1a_dup, w1b=w1b_dup,
                  w2bd=w2bd.astype(_BF), w3bd=w3bd.astype(_BF),
                  id128=id128, hsel=hsel.astype(_BF), bq128=bq128, bk128=bk128,
                  b1d=b1d, b2d=b2d, b3t=b3t)
    maps = []
    for c in range(NCORES):
        b, half = c // 2, c % 2
        rows = slice(half * NQ, (half + 1) * NQ)
        m = dict(shared)
        m["qT"] = np.ascontiguousarray(q[b, rows].T).astype(_BF)
        m["kT"] = np.ascontiguousarray(k[b].T).astype(_BF)
        m["vTP"] = pack_w(np.ascontiguousarray(v[b].T), 4).astype(_BF)
        m["seT"] = np.ascontiguousarray(se[b].T)
        m["seQ"] = np.ascontiguousarray(se[b, rows].T)
        maps.append(m)
    return maps


_cache = {}


def _get_program():
    if "nc" not in _cache:
        _cache["nc"] = _build_program()
    return _cache["nc"]


def kernel(**inputs):
    nc = _get_program()
    maps = _host_prep(inputs)
    res = run_bass_kernel_spmd(nc, maps, list(range(NCORES)))
    # attention weights sum to 1, so bv contributes exactly bv @ Wo
    bo = (np.asarray(inputs["bo"], np.float32)
          + np.asarray(inputs["bv"], np.float32)
          @ np.asarray(inputs["Wo"], np.float32))
    out = np.empty((B, N, E), np.float32)
    for c in range(NCORES):
        b, half = c // 2, c % 2
        out[b, half * NQ:(half + 1) * NQ] = res.results[c]["out"]
    return out + bo


# revision 38
# speedup vs baseline: 1.1632x; 1.0277x over previous
"""CAB multi-head attention on 8 Trainium2 NeuronCores.

Sharding: fully data-parallel, core c -> (batch b = c//2, query-half = c%2).
Each core computes 256 query rows against all 512 keys of its batch.
No collectives. Host does transposes/packing; device does all FLOPs.

Per-core layout conventions (features on partitions, tokens on free):
  QT/KT [E, t] f32r; V [s, e] f32r; scoresT/attnT [s, t] (softmax along
  partitions via one-hot-column matmuls, no max subtraction needed);
  CAB pairs i-major: h [(d, i%2), j] packed 2 queries per [128, 1024]
  tile; comp [(iic, h, i%2), j] is PE-transposed into the tt-major
  biasT [j, (tt, jc, iic, h, m)] with ONE contiguous scatter copy per
  tt; stage C reads bias via strided 3-dim APs (one per jc).

Engine plan (v3): stage B is elementwise-bound; relu1 runs on the DVE
(bf16 2x mode, 327ns), relu2 on the ACT as [128, 1024] double-tiles
(997ns per 2 queries), the W3 scale and the bias scatter alternate.
Head temps are folded into the W3 weights on the host so the scale op
is a single-ALU-op tensor_scalar.  All V-projection jobs moved into
stage B (PE slack); stage C keeps only the head-10..15 Q/K chunks.
"""
import sys

sys.path.insert(0, "/opt/trn_rl_repo")

import numpy as np
import ml_dtypes
from contextlib import ExitStack

import concourse.bacc as bacc
import concourse.tile as tile
from concourse import mybir
from concourse.bass_utils import run_bass_kernel_spmd

F32 = mybir.dt.float32
F32R = mybir.dt.float32r
BF16 = mybir.dt.bfloat16
AF = mybir.ActivationFunctionType
ALU = mybir.AluOpType

B, N, E, H, SD, HID = 4, 512, 1024, 16, 64, 64
D = E // H
NQ = 256            # query rows per core
NCORES = 8
NTT = NQ // 8       # 32 tt groups (4 i-pairs each) in the CAB stage

_BF = ml_dtypes.bfloat16


def _build_program(debug=False):
    nc = bacc.Bacc("TRN2", target_bir_lowering=False, debug=False,
                   num_devices=NCORES)

    def din(name, shape, dt):
        return nc.dram_tensor(name, list(shape), dt, kind="ExternalInput").ap()

    d = {}
    d["qT"] = din("qT", (E, NQ), BF16)
    d["kT"] = din("kT", (E, N), BF16)
    d["seT"] = din("seT", (SD, N), F32R)
    d["seQ"] = din("seQ", (SD, NQ), F32R)
    # wqP/wkP/vTP are host-packed so each per-job SBUF tile is one
    # contiguous 256KB row-slice (cheap DMA descriptor generation):
    # row ec*128+p, col k*128+c  <-  w[k*128+p, ec*128+c]
    d["wqP"] = din("wqP", (E, E), BF16)
    d["wkP"] = din("wkP", (E, E), BF16)
    d["vTP"] = din("vTP", (N, E), BF16)
    d["wv"] = din("wv", (E, E), BF16)
    d["wo"] = din("wo", (E, E), BF16)
    d["w1a"] = din("w1a", (SD, 128), F32R)
    d["w1b"] = din("w1b", (SD, 128), F32R)
    d["w2bd"] = din("w2bd", (128, 128), BF16)
    d["w3bd"] = din("w3bd", (128, 32), BF16)
    d["id128"] = din("id128", (128, 128), BF16)
    d["hsel"] = din("hsel", (128, 32), BF16)
    d["bq128"] = din("bq128", (128, 8), F32)
    d["bk128"] = din("bk128", (128, 8), F32)
    d["b1d"] = din("b1d", (128, 1), F32)
    d["b2d"] = din("b2d", (128, 1), F32)
    d["b3t"] = din("b3t", (128, 1), F32)
    out_d = nc.dram_tensor("out", [NQ, E], F32, kind="ExternalOutput").ap()
    rscratch = nc.dram_tensor("rscratch", [16, NQ], F32,
                              kind="ExternalOutput").ap()

    with tile.TileContext(nc) as tc, ExitStack() as ctx:
        # ---------------- persistent SBUF pools ----------------
        cst = ctx.enter_context(tc.tile_pool(name="cst", bufs=1))
        big = ctx.enter_context(tc.tile_pool(name="big", bufs=1))

        def cload(name, shape, dt, eng=None):
            eng = eng or nc.sync
            t = cst.tile(list(shape), dt, tag=name, name=name)
            eng.dma_start(t[:], d[name][:])
            return t

        # Startup DMA descriptor generation (~0.7us per dma_start) is spread
        # across all five engine queues so the stage-A/B critical loads
        # aren't serialized behind bulk input loads.
        seT = cload("seT", (SD, N), F32R)                     # sync
        w1b = cload("w1b", (SD, 128), F32R)
        b1d = cload("b1d", (128, 1), F32)
        # prefetch the first 4 k_job weight blocks on the sync queue so the
        # first jobs never head-of-line-block the in-order PE queue
        wkpre = []
        for ec in range(4):
            t = cst.tile([128, 1024], BF16, tag=f"wkpre{ec}", name=f"wkpre{ec}")
            nc.sync.dma_start(t[:], d["wkP"][ec * 128:(ec + 1) * 128, :])
            wkpre.append(t)
        w1a = cload("w1a", (SD, 128), F32R, eng=nc.gpsimd)
        seQ = cload("seQ", (SD, NQ), F32R, eng=nc.gpsimd)
        w2bd = cload("w2bd", (128, 128), BF16, eng=nc.gpsimd)
        b2d = cload("b2d", (128, 1), F32, eng=nc.gpsimd)
        id128 = cload("id128", (128, 128), BF16, eng=nc.gpsimd)
        w3bd = cload("w3bd", (128, 32), BF16, eng=nc.gpsimd)
        b3t = cload("b3t", (128, 1), F32, eng=nc.gpsimd)
        bq128 = cload("bq128", (128, 8), F32, eng=nc.gpsimd)
        bk128 = cload("bk128", (128, 8), F32, eng=nc.gpsimd)
        hsel = cload("hsel", (128, 32), BF16, eng=nc.gpsimd)

        # resident per-core inputs, chunked on k (one DMA each, k-chunk kc
        # of a [E, t] tensor lives in tile kc as [128, t]).
        def kchunks(name, t, dt, ntile=8, eng=None):
            eng = eng or nc.sync
            ts = []
            for k in range(ntile):
                tt = big.tile([128, t], dt, tag=f"{name}{k}", name=f"{name}{k}")
                eng.dma_start(tt[:], d[name][k * 128:(k + 1) * 128, :])
                ts.append(tt)
            return ts

        kTt = kchunks("kT", N, BF16)
        # Wv rows resident (rhs of V-proj), Wo rows resident (rhs of out-proj)
        wv_r = kchunks("wv", E, BF16, eng=nc.gpsimd)
        qTt = kchunks("qT", NQ, BF16, eng=nc.gpsimd)
        # wo is needed only from stage C on; its DMAs are deferred into the
        # tt loop so they don't eat startup HBM bandwidth (which delays the
        # first k_job weights and head-of-line-blocks the PE/ACT queues)
        wo_r = [big.tile([128, E], BF16, tag=f"wo{k}", name=f"wo{k}")
                for k in range(8)]

        # persistent intermediates
        QT = [big.tile([128, NQ], BF16, tag=f"QT{k}", name=f"QT{k}") for k in range(8)]
        KT = [big.tile([128, N], BF16, tag=f"KT{k}", name=f"KT{k}") for k in range(8)]
        Vsb = [[big.tile([128, 512], BF16, tag=f"V{st}_{et}", name=f"V{st}_{et}")
                for et in range(2)] for st in range(4)]
        hjT = big.tile([128, N], BF16, tag="hjT")
        hiT = big.tile([128, 128], F32, tag="hiT")
        # biasT free layout (tt-major): tt*512 + jc*128 + iic*32 + h*2 + m,
        # partition = j within chunk jc.  Written by ONE contiguous copy per
        # tt; stage C reads strided [jc-slices] per (h, half).
        biasT = big.tile([128, NTT * 512], BF16, tag="biasT")
        avN = [big.tile([128, NQ], BF16, tag=f"avN{hp}", name=f"avN{hp}") for hp in range(8)]

        # ---------------- stage A: W1 (tiny) ----------------
        with tc.tile_pool(name="w1ps", bufs=1, space="PSUM") as w1ps:
            hj_ps = w1ps.tile([128, N], F32, tag="hjps")
            nc.tensor.matmul(hj_ps[:], w1b[:], seT[:], start=True, stop=True)
            nc.scalar.activation(hjT[:], hj_ps[:], AF.Identity,
                                 bias=b1d[:, 0:1])
            hi_ps = w1ps.tile([128, NQ], F32, tag="hips")
            nc.tensor.matmul(hi_ps[:], w1a[:], seQ[:], start=True, stop=True)
            hi_v = hi_ps[:].rearrange("p (i two) -> p i two", two=2)
            nc.vector.tensor_copy(hiT[0:64, :], hi_v[0:64, :, 0])
            nc.vector.tensor_copy(hiT[64:128, :], hi_v[64:128, :, 1])

        # ---------------- stage B: CAB pair-MLP + QKV projections ----------
        with tc.tile_pool(name="wcol", bufs=4) as wcol, \
             tc.tile_pool(name="p1ps", bufs=1, space="PSUM") as p1ps, \
             tc.tile_pool(name="hpool", bufs=3) as hpool, \
             tc.tile_pool(name="h2sb", bufs=3) as h2sbp, \
             tc.tile_pool(name="csb", bufs=3) as csbp, \
             tc.tile_pool(name="h2ps", bufs=2, space="PSUM") as h2ps, \
             tc.tile_pool(name="cps", bufs=2, space="PSUM") as cps, \
             tc.tile_pool(name="trps", bufs=1, space="PSUM") as trps:

            # ---- projection jobs, interleaved through the tt loop ----
            # (head-chunks 5-7 of Q/K are only consumed by stage-C pairs 5-7,
            # so those jobs run inside stage C to keep its PE un-throttled)
            def q_job(ec, wpool, pspool):
                wq_c = wpool.tile([128, 1024], BF16, tag="wcol")
                nc.sync.dma_start(
                    wq_c[:], d["wqP"][ec * 128:(ec + 1) * 128, :])
                ps = pspool.tile([128, 512], F32, tag="p1", name="qps")[:, 0:NQ]
                for kc in range(8):
                    nc.tensor.matmul(ps[:], wq_c[:, kc * 128:(kc + 1) * 128],
                                     qTt[kc][:], start=(kc == 0),
                                     stop=(kc == 7))
                nc.vector.tensor_scalar(QT[ec][:], ps[:],
                                        bq128[:, ec:ec + 1], None, ALU.add)

            def k_job(ec, wpool, pspool):
                if ec < 4:
                    wk_c = wkpre[ec]
                else:
                    wk_c = wpool.tile([128, 1024], BF16, tag="wcol")
                    nc.sync.dma_start(
                        wk_c[:], d["wkP"][ec * 128:(ec + 1) * 128, :])
                ps = pspool.tile([128, 512], F32, tag="p1", name="kvps")
                for kc in range(8):
                    nc.tensor.matmul(ps[:], wk_c[:, kc * 128:(kc + 1) * 128],
                                     kTt[kc][:], start=(kc == 0),
                                     stop=(kc == 7))
                nc.scalar.activation(KT[ec][:], ps[:],
                                     AF.Identity, bias=bk128[:, ec:ec + 1])

            def v_job(st, et, wpool=None, pspool=None):
                wpool = wpool or wcol
                pspool = pspool or p1ps
                vt_c = wpool.tile([128, 1024], BF16, tag="vtcb")
                nc.sync.dma_start(
                    vt_c[:], d["vTP"][st * 128:(st + 1) * 128, :])
                ps = pspool.tile([128, 512], F32, tag="p1", name="kvps")
                for kc in range(8):
                    nc.tensor.matmul(
                        ps[:], vt_c[:, kc * 128:(kc + 1) * 128],
                        wv_r[kc][:, et * 512:(et + 1) * 512],
                        start=(kc == 0), stop=(kc == 7))
                if st % 2 == 0:
                    nc.vector.tensor_copy(Vsb[st][et][:], ps[:])
                else:
                    nc.scalar.copy(Vsb[st][et][:], ps[:])

            jobs = ([lambda ec=ec: k_job(ec, wcol, p1ps) for ec in range(5)]
                    + [lambda ec=ec: q_job(ec, wcol, p1ps) for ec in range(5)]
                    + [lambda st=st: v_job(st, 0) for st in range(4)]
                    + [lambda st=st: v_job(st, 1) for st in range(4)])
            njobs = len(jobs)
            job_i = 0

            for tt in range(NTT):
                if tt == 8:
                    for k in range(8):
                        nc.gpsimd.dma_start(
                            wo_r[k][:], d["wo"][k * 128:(k + 1) * 128, :])
                # spread the 18 projection jobs over tt 3..31 (job weights
                # need a few us of startup DMA bandwidth to arrive)
                while job_i < njobs and job_i < max(0, tt - 2) * njobs // (NTT - 3):
                    jobs[job_i]()
                    job_i += 1

                h2_tiles = []
                for pr in range(2):
                    h_t = hpool.tile([128, 2 * N], BF16, tag="h")
                    for k in range(2):
                        ii = tt * 4 + pr * 2 + k
                        nc.vector.tensor_scalar(h_t[:, k * N:(k + 1) * N],
                                                hjT[:], hiT[:, ii:ii + 1],
                                                0.0, ALU.add, ALU.max)
                    ps = h2ps.tile([128, 2 * N], F32, tag="h2")
                    for k in range(2):
                        nc.tensor.matmul(ps[:, k * N:(k + 1) * N], w2bd[:],
                                         h_t[:, k * N:(k + 1) * N],
                                         start=True, stop=True)
                    h2_t = h2sbp.tile([128, 2 * N], BF16, tag="h2sb")
                    nc.scalar.activation(h2_t[:], ps[:], AF.Relu,
                                         bias=b2d[:, 0:1])
                    h2_tiles.append(h2_t)

                c_ps = cps.tile([128, N], F32, tag="comp")
                for iic in range(4):
                    nc.tensor.matmul(c_ps[32 * iic:32 * iic + 32, :],
                                     w3bd[:],
                                     h2_tiles[iic // 2][:, (iic % 2) * N:
                                                        (iic % 2 + 1) * N],
                                     start=True, stop=True,
                                     tile_position=(0, 32 * iic))
                # temps are folded into w3bd on the host; single add + cast
                c_sb = csbp.tile([128, N], BF16, tag="csb")
                nc.vector.tensor_scalar(c_sb[:], c_ps[:], b3t[:, 0:1],
                                        None, ALU.add)
                tr_ps = trps.tile([128, 512], BF16, tag="tr")
                for jc in range(4):
                    nc.tensor.transpose(tr_ps[:, jc * 128:(jc + 1) * 128],
                                        c_sb[:, jc * 128:(jc + 1) * 128],
                                        id128[:])
                # ONE contiguous scatter copy into the tt-major biasT
                if tt % 2 == 0:
                    nc.scalar.copy(biasT[:, tt * 512:(tt + 1) * 512], tr_ps[:])
                else:
                    nc.vector.tensor_copy(biasT[:, tt * 512:(tt + 1) * 512],
                                          tr_ps[:])

        # ---------------- stage C: scores + softmax + AV ----------------
        # Software-pipelined head loop: head h+1's scores/bias/exp are
        # emitted BEFORE head h's sums/AV matmuls, so the in-order PE queue
        # never stalls on the exp.  scps bufs=4 holds exactly 2 heads.
        # Stage-D partial accumulation for out-block (0,0) replaces the
        # warm_mm filler on the job-less pairs 6-7.
        # bias view: [j-part, (h, jc, tt, iic, m)]
        bT5 = biasT[:].rearrange("p (t j i x m) -> p x j t i m",
                                 t=NTT, j=4, i=4, x=16, m=2)
        with tc.tile_pool(name="attnT", bufs=5) as attp, \
             tc.tile_pool(name="vcol", bufs=4) as vcol, \
             tc.tile_pool(name="vps", bufs=1, space="PSUM") as vps, \
             tc.tile_pool(name="r2sb", bufs=2) as r2sb, \
             tc.tile_pool(name="rc2", bufs=2) as rc2p, \
             tc.tile_pool(name="osb", bufs=2) as osb:

            # per-pair long-matmul jobs: the Q/K projection chunks for heads
            # 10-15 (each needed only from its own pair on)
            cjobs = {
                0: [lambda: k_job(5, vcol, vps)],
                1: [lambda: q_job(5, vcol, vps)],
                2: [lambda: k_job(6, vcol, vps)],
                3: [lambda: q_job(6, vcol, vps)],
                4: [lambda: k_job(7, vcol, vps)],
                5: [lambda: q_job(7, vcol, vps)],
            }
            dps_t = None

            # scps bufs=5 gives ~3 halves of PE lookahead; the softmax sums
            # share the av PSUM bank (avsums cols 256:512 on partitions 0:2)
            # so only one av/sums tile per pair is needed.  The pair's very
            # first AV matmul carries the lone start=True (clears the bank's
            # has_written bits); every other matmul overwrites/accumulates
            # via those bits in program order.
            with tc.tile_pool(name="scps", bufs=3, space="PSUM") as scps, \
                 tc.tile_pool(name="smps", bufs=1, space="PSUM") as smps, \
                 tc.tile_pool(name="avps", bufs=3, space="PSUM") as avps:

                def emit_half(h, half):
                    hp, hw = h // 2, (h % 2) * 64
                    sc_ps = scps.tile([128, 512], F32, tag="sc")
                    for q in range(2):
                        jc = half * 2 + q
                        nc.tensor.matmul(
                            sc_ps[:, q * 256:(q + 1) * 256],
                            KT[hp][hw:hw + 64, jc * 128:(jc + 1) * 128],
                            QT[hp][hw:hw + 64, :],
                            start=True, stop=True, skip_group_check=True)
                    for q in range(2):
                        jc = half * 2 + q
                        nc.vector.tensor_tensor(
                            sc_ps[:, q * 256:(q + 1) * 256],
                            sc_ps[:, q * 256:(q + 1) * 256],
                            bT5[:, h, jc], ALU.add)
                    at = attp.tile([128, 512], BF16, tag="at")
                    nc.scalar.activation(at[:], sc_ps[:], AF.Exp)
                    return at

                LOOK = 2
                pend = {}
                avsums = None
                sums2 = None
                for idx in range(32 + LOOK):
                    if idx < 32:
                        h, half = idx // 2, idx % 2
                        if idx % 4 == 0:
                            hp = h // 2
                            for job in cjobs.get(hp, []):
                                job()
                            if hp == 6:
                                dps_t = vps.tile([128, 512], F32, tag="p1",
                                                 name="dpart")
                                for ihp in range(4):
                                    nc.tensor.matmul(
                                        dps_t[:], avN[ihp][:, 0:128],
                                        wo_r[ihp][:, 0:512], start=(ihp == 0),
                                        stop=False, skip_group_check=True)
                            elif hp == 7:
                                for ihp in (4, 5):
                                    nc.tensor.matmul(
                                        dps_t[:], avN[ihp][:, 0:128],
                                        wo_r[ihp][:, 0:512], start=False,
                                        stop=False, skip_group_check=True)
                        pend[idx] = emit_half(h, half)
                    j = idx - LOOK
                    if j < 0:
                        continue
                    h, half = j // 2, j % 2
                    hp, hw = h // 2, (h % 2) * 64
                    if j % 4 == 0:
                        avsums = avps.tile([128, NQ], F32, tag="av")
                        sums2 = smps.tile([2, NQ], F32, tag="s2")
                    ats = pend.pop(j)
                    for q in range(2):
                        jc = half * 2 + q
                        atv = ats[:, q * 256:(q + 1) * 256]
                        nc.tensor.matmul(
                            avsums[hw:hw + 64, :],
                            Vsb[jc][h // 8][:, (h % 8) * 64:(h % 8) * 64 + 64],
                            atv,
                            start=(half == 0 and q == 0),
                            stop=(half == 1 and q == 1),
                            skip_group_check=True,
                            tile_position=(0, hw))
                    for q in range(2):
                        jc = half * 2 + q
                        atv = ats[:, q * 256:(q + 1) * 256]
                        nc.tensor.matmul(
                            sums2[:], hsel[:, 2 * h:2 * h + 2], atv,
                            start=(j % 4 == 0 and q == 0),
                            stop=(j % 4 == 3 and q == 1),
                            skip_group_check=True)
                    if j % 4 == 3:
                        recip2 = rc2p.tile([2, NQ], F32, tag="rc2")
                        nc.vector.reciprocal_approx_fast(recip2[:], sums2[:])
                        nc.sync.dma_start(rscratch[2 * hp:2 * hp + 2, :],
                                          recip2[:])
                        r2 = r2sb.tile([128, NQ], F32, tag="r2")
                        rsrc = rscratch[2 * hp:2 * hp + 2, :].rearrange(
                            "h (o t) -> h o t", o=1)
                        nc.sync.dma_start(r2[:], rsrc.broadcast_to([2, 64, NQ]))
                        nc.vector.tensor_tensor(avN[hp][:], avsums[:],
                                                r2[:], ALU.mult)

            # ------------- stage D: output projection -------------
            # (scps/smps/avps banks are freed; block (0,0) finishes on the
            # vps bank it accumulated into during pairs 6-7)
            with tc.tile_pool(name="ops", bufs=2, space="PSUM") as ops:
                for ihp in (6, 7):
                    nc.tensor.matmul(dps_t[:], avN[ihp][:, 0:128],
                                     wo_r[ihp][:, 0:512], start=False,
                                     stop=(ihp == 7), skip_group_check=True)
                o_sb = osb.tile([128, 512], F32, tag="osb")
                nc.scalar.copy(o_sb[:], dps_t[:])
                nc.sync.dma_start(out_d[0:128, 0:512], o_sb[:])
                for ttile, et in ((0, 1), (1, 0), (1, 1)):
                    ps = ops.tile([128, 512], F32, tag="ops")
                    for hp in range(8):
                        nc.tensor.matmul(
                            ps[:], avN[hp][:, ttile * 128:(ttile + 1) * 128],
                            wo_r[hp][:, et * 512:(et + 1) * 512],
                            start=(hp == 0), stop=(hp == 7))
                    o_sb = osb.tile([128, 512], F32, tag="osb")
                    nc.scalar.copy(o_sb[:], ps[:])
                    nc.sync.dma_start(
                        out_d[ttile * 128:(ttile + 1) * 128,
                              et * 512:(et + 1) * 512], o_sb[:])

    nc.compile()
    return nc


def _host_prep(inputs):
    """Build the 8 per-core input maps from the full inputs."""
    f32 = np.float32
    q = np.ascontiguousarray(inputs["query"], f32)
    k = np.ascontiguousarray(inputs["key"], f32)
    v = np.ascontiguousarray(inputs["value"], f32)
    se = np.ascontiguousarray(inputs["state_embeddings"], f32)
    scale = f32(D) ** f32(-0.5)
    # packed layouts: row ec*128+p, col k*128+c  <-  w[k*128+p, ec*128+c]
    def pack_w(w, necol):
        return np.ascontiguousarray(
            w.reshape(8, 128, necol, 128).transpose(2, 1, 0, 3)
            .reshape(necol * 128, 1024))
    wqP = pack_w(np.asarray(inputs["Wq"], f32) * scale, 8).astype(_BF)
    wkP = pack_w(np.asarray(inputs["Wk"], f32), 8).astype(_BF)
    wv = np.ascontiguousarray(inputs["Wv"]).astype(_BF)
    wo = np.ascontiguousarray(inputs["Wo"]).astype(_BF)
    bq = np.asarray(inputs["bq"], f32) * scale
    bk = np.asarray(inputs["bk"], f32)
    w1 = np.asarray(inputs["W1"], f32)
    b1 = np.asarray(inputs["b1"], f32)
    w2 = np.asarray(inputs["W2"], f32)
    b2 = np.asarray(inputs["b2"], f32)
    w3 = np.asarray(inputs["W3"], f32)
    b3 = np.asarray(inputs["b3"], f32)
    temps = np.asarray(inputs["head_temps"], f32)

    w1a_dup = np.concatenate([w1[:SD], w1[:SD]], axis=1)          # [64,128]
    w1b_dup = np.concatenate([w1[SD:], w1[SD:]], axis=1)          # [64,128]
    w2bd = np.zeros((128, 128), f32)
    w2bd[:64, :64] = w2
    w2bd[64:, 64:] = w2
    # head temps folded into the W3 columns (m = 2*h + par, h-major pairs)
    w3t = w3 * temps[None, :]
    w3bd = np.zeros((128, 32), f32)
    w3bd[:64, 0::2] = w3t
    w3bd[64:, 1::2] = w3t
    hsel = np.zeros((128, 32), f32)
    for h in range(H):
        hsel[:, 2 * h + h % 2] = 1.0
    hidx = (np.arange(128) % 32) // 2
    b3t = (b3 * temps)[hidx].reshape(128, 1)
    b1d = np.tile(b1, 2).reshape(128, 1)
    b2d = np.tile(b2, 2).reshape(128, 1)
    bq128 = bq.reshape(8, 128).T.copy()
    bk128 = bk.reshape(8, 128).T.copy()
    id128 = np.eye(128, dtype=f32).astype(_BF)

    shared = dict(wqP=wqP, wkP=wkP, wv=wv, wo=wo, w1a=w1a_dup, w1b=w1b_dup,
                  w2bd=w2bd.astype(_BF), w3bd=w3bd.astype(_BF),
                  id128=id128, hsel=hsel.astype(_BF), bq128=bq128, bk128=bk128,
                  b1d=b1d, b2d=b2d, b3t=b3t)
    maps = []
    for c in range(NCORES):
        b, half = c // 2, c % 2
        rows = slice(half * NQ, (half + 1) * NQ)
        m = dict(shared)
        m["qT"] = np.ascontiguousarray(q[b, rows].T).astype(_BF)
        m["kT"] = np.ascontiguousarray(k[b].T).astype(_BF)
        m["vTP"] = pack_w(np.ascontiguousarray(v[b].T), 4).astype(_BF)
        m["seT"] = np.ascontiguousarray(se[b].T)
        m["seQ"] = np.ascontiguousarray(se[b, rows].T)
        maps.append(m)
    return maps


_cache = {}


def _get_program():
    if "nc" not in _cache:
        _cache["nc"] = _build_program()
    return _cache["nc"]


def kernel(**inputs):
    nc = _get_program()
    maps = _host_prep(inputs)
    res = run_bass_kernel_spmd(nc, maps, list(range(NCORES)))
    # attention weights sum to 1, so bv contributes exactly bv @ Wo
    bo = (np.asarray(inputs["bo"], np.float32)
          + np.asarray(inputs["bv"], np.float32)
          @ np.asarray(inputs["Wo"], np.float32))
    out = np.empty((B, N, E), np.float32)
    for c in range(NCORES):
        b, half = c // 2, c % 2
        out[b, half * NQ:(half + 1) * NQ] = res.results[c]["out"]
    return out + bo
